# revision 1
# baseline (speedup 1.0000x reference)
"""Trainium2 Bass kernel for nn_DemLocGraphEncoder (4-layer GIN + variational heads).

Strategy
--------
The GIN segment-sum aggregation is recast as a dense matmul with a
host-precomputed (I + A)^T adjacency-multiplicity matrix (N=8192, so the
dense form maps perfectly onto the 128x128 TensorEngine; avg degree 32
makes gather/scatter paths no faster and far more complex).

Sharding: nodes are row-sharded 1024/core across 8 cores.  Each layer:
  1. AllGather node features x (node-major) -> x_full  [skipped for layer 0,
     whose input is replicated to every core]
  2. agg^T = x_full^T @ ATshard  on TensorE (feature-major output)
  3. MLP entirely in feature-major form: hT = relu(W1-matmuls + b1),
     xT = relu(W2-matmuls + b2)  (weights replicated, used directly as lhsT)
  4. PE-transpose xT -> node-major x_own, DMA to DRAM for the next AllGather.
Layer 3 folds w2_3 @ {wm,wv} into two fused [2048,128] heads (x3 is never
materialized), then z = mean + var*eps on the VectorEngine.

All matmuls run in fp16 (1 cycle/row on TRN2 PE, fp32 PSUM accumulation;
fp16 chosen over bf16 for its 10-bit mantissa; activations stay < ~1e3 so
no overflow).  Outputs are fp32.
"""

import sys

if "/opt/trn_rl_repo" not in sys.path:
    sys.path.insert(0, "/opt/trn_rl_repo")

import numpy as np

N, E, T, H, O, L = 8192, 262144, 256, 2048, 1024, 128
NC = 8
NS = N // NC          # 1024 nodes per core
P = 128
KT_NODES = N // P     # 64 k-tiles over source nodes
ND = NS // 512        # 2 free-dim tiles over own nodes

_PROGRAM_CACHE = {}


def _build_program(collectives=True, opts=None):
    opts = dict(opts or {})
    no_transpose = opts.get("no_transpose", False)   # sim-only: DMA instead of PE transpose
    drain_split = opts.get("drain_split", True)     # alternate agg drains DVE/ACT
    at_bufs = opts.get("at_bufs", 14)
    x_bufs = opts.get("x_bufs", 7)
    w_bufs = opts.get("w_bufs", 6)
    ps_bufs = opts.get("ps_bufs", 8)
    agg_group = opts.get("agg_group", 8)
    dma_tp = opts.get("dma_tp", False)     # feature-major AG + XBAR-transposed x loads
    l0_split = opts.get("l0_split", True)  # pipeline layer-0 n-halves with MLP
    import concourse.bass as bass  # noqa: F401
    import concourse.mybir as mybir
    import concourse.tile as tile
    from concourse import bacc
    from concourse.masks import make_identity

    f16 = mybir.dt.float16
    f32 = mybir.dt.float32
    AF = mybir.ActivationFunctionType

    nc = bacc.Bacc(
        "TRN2", target_bir_lowering=False, debug=False,
        num_devices=NC if collectives else 1,
    )

    # ---- I/O ----
    at_d = nc.dram_tensor("at_t", [KT_NODES, ND, P, 512], f16, kind="ExternalInput")
    x0_d = nc.dram_tensor("x0", [KT_NODES, P, T], f16, kind="ExternalInput")
    w_d = {}
    w_d["w1_0"] = nc.dram_tensor("w1_0", [H // P, P, T // P, P], f16, kind="ExternalInput")
    w_d["w2_0"] = nc.dram_tensor("w2_0", [H // P, P, H // P, P], f16, kind="ExternalInput")
    for l in (1, 2):
        w_d[f"w1_{l}"] = nc.dram_tensor(f"w1_{l}", [H // P, P, H // P, P], f16, kind="ExternalInput")
    w_d["w1_3"] = nc.dram_tensor("w1_3", [O // P, P, H // P, P], f16, kind="ExternalInput")
    for l in (1, 2):
        w_d[f"w2_{l}"] = nc.dram_tensor(f"w2_{l}", [H // P, P, H // P, P], f16, kind="ExternalInput")
    whm_d = nc.dram_tensor("whm", [P, O // P, P], f16, kind="ExternalInput")
    whv_d = nc.dram_tensor("whv", [P, O // P, P], f16, kind="ExternalInput")
    b_d = {}
    for l in range(3):
        b_d[f"b1_{l}"] = nc.dram_tensor(f"b1_{l}", [P, H // P], f32, kind="ExternalInput")
    b_d["b1_3"] = nc.dram_tensor("b1_3", [P, O // P], f32, kind="ExternalInput")
    for l in range(3):
        b_d[f"b2_{l}"] = nc.dram_tensor(f"b2_{l}", [P, H // P], f32, kind="ExternalInput")
    bhm_d = nc.dram_tensor("bhm", [P, 1], f32, kind="ExternalInput")
    bhv_d = nc.dram_tensor("bhv", [P, 1], f32, kind="ExternalInput")
    eps_d = nc.dram_tensor("epst", [P, NS], f32, kind="ExternalInput")

    z_d = nc.dram_tensor("zt", [P, NS], f32, kind="ExternalOutput")
    mean_d = nc.dram_tensor("meant", [P, NS], f32, kind="ExternalOutput")
    var_d = nc.dram_tensor("vart", [P, NS], f32, kind="ExternalOutput")

    HH = H // 2
    if dma_tp:
        # feature-major: xown [HH feats, NS nodes]; gathered [NC*HH, NS]
        xown = {(l, h): nc.dram_tensor(f"xown{l}_{h}", [HH, NS], f16)
                for l in (1, 2, 3) for h in (0, 1)}
        xg = {(l, h): nc.dram_tensor(f"xg{l}_{h}", [NC * HH, NS], f16, addr_space="Shared")
              for l in (1, 2, 3) for h in (0, 1)}
    else:
        xown = {(l, h): nc.dram_tensor(f"xown{l}_{h}", [NS, HH], f16)
                for l in (1, 2, 3) for h in (0, 1)}
        xg = {(l, h): nc.dram_tensor(f"xg{l}_{h}", [N, HH], f16, addr_space="Shared")
              for l in (1, 2, 3) for h in (0, 1)}

    rg = [list(range(NC))]

    with tile.TileContext(nc) as tc:
        with (
            tc.tile_pool(name="const", bufs=1) as const_p,
            tc.tile_pool(name="big", bufs=1) as big_p,
            tc.tile_pool(name="at", bufs=at_bufs) as at_p,
            tc.tile_pool(name="xslab", bufs=x_bufs) as x_p,
            tc.tile_pool(name="w", bufs=w_bufs) as w_p,
            tc.tile_pool(name="xo", bufs=2) as xo_p,
            tc.tile_pool(name="ps", bufs=ps_bufs, space="PSUM") as ps_p,
        ):
            ident = const_p.tile([P, P], f16, tag="ident")
            make_identity(nc, ident)

            bias_sb = {}
            for name, d in b_d.items():
                bias_sb[name] = const_p.tile(list(d.shape), f32, tag=f"b_{name}", name=f"b_{name}")
                nc.sync.dma_start(bias_sb[name][:], d[:])
            bhm_sb = const_p.tile([P, 1], f32, tag="bhm")
            nc.sync.dma_start(bhm_sb[:], bhm_d[:])
            bhv_sb = const_p.tile([P, 1], f32, tag="bhv")
            nc.sync.dma_start(bhv_sb[:], bhv_d[:])
            eps_sb = const_p.tile([P, NS], f32, tag="eps")
            nc.sync.dma_start(eps_sb[:], eps_d[:])
            whm_sb = const_p.tile([P, O // P, P], f16, tag="whm")
            nc.sync.dma_start(whm_sb[:], whm_d[:])
            whv_sb = const_p.tile([P, O // P, P], f16, tag="whv")
            nc.sync.dma_start(whv_sb[:], whv_d[:])

            def all_gather(l, h):
                if collectives:
                    nc.gpsimd.collective_compute(
                        "AllGather", mybir.AluOpType.bypass, replica_groups=rg,
                        ins=[xown[l, h][:].opt()], outs=[xg[l, h][:].opt()],
                    )
                else:
                    # sim-only stand-in: model the DMA traffic of the gather
                    for c in range(NC):
                        nc.sync.dma_start(xg[l, h][c * NS:(c + 1) * NS, :], xown[l, h][:])

            def agg(d_in, x_load_fn, uT, ns=None):
                """uT[:, mt, n*512:(n+1)*512] = sum_k x[k,m]^T @ AT[k,n]."""
                Mt = d_in // P
                for n in (range(ND) if ns is None else ns):
                    for g0 in range(0, Mt, agg_group):
                        gsz = min(agg_group, Mt - g0)
                        psums = [ps_p.tile([P, 512], f32, tag="mm", name=f"ps{_i}") for _i in range(gsz)]
                        for k in range(KT_NODES):
                            xs = x_p.tile([P, gsz * P], f16, tag="xslab")
                            x_load_fn(xs, k, g0 * P, gsz * P)
                            att = at_p.tile([P, 512], f16, tag="at")
                            nc.sync.dma_start(att[:], at_d[k, n])
                            for mi in range(gsz):
                                nc.tensor.matmul(
                                    psums[mi][:],
                                    lhsT=xs[:, mi * P:(mi + 1) * P],
                                    rhs=att[:],
                                    start=(k == 0),
                                    stop=(k == KT_NODES - 1),
                                )
                        for mi in range(gsz):
                            dst = uT[:, g0 + mi, n * 512:(n + 1) * 512]
                            if drain_split and mi % 2 == 1:
                                nc.scalar.copy(dst, psums[mi][:])
                            else:
                                nc.vector.tensor_copy(dst, psums[mi][:])

            def linear(w_dram, Kt, Mt, rhsT, outT, bias, relu, out_off=0, mts=None, ns=None):
                for mt in (range(Mt) if mts is None else mts):
                    ws = w_p.tile([P, Kt, P], f16, tag="w")
                    nc.sync.dma_start(ws[:], w_dram[mt])
                    for n in (range(ND) if ns is None else ns):
                        p = ps_p.tile([P, 512], f32, tag="mm")
                        for k in range(Kt):
                            nc.tensor.matmul(
                                p[:],
                                lhsT=ws[:, k, :],
                                rhs=rhsT[:, k, n * 512:(n + 1) * 512],
                                start=(k == 0),
                                stop=(k == Kt - 1),
                            )
                        nc.scalar.activation(
                            outT[:, out_off + mt, n * 512:(n + 1) * 512],
                            p[:],
                            AF.Relu if relu else AF.Identity,
                            bias=bias[:, mt:mt + 1],
                        )

            def transpose_store(xT, xown_dram, half):
                mt0 = half * (H // P // 2)
                nmt = H // P // 2
                if dma_tp:
                    # store feature-major directly; transposition happens on the
                    # post-AllGather XBAR load
                    for mt in range(nmt):
                        nc.sync.dma_start(
                            xown_dram[mt * P:(mt + 1) * P, :], xT[:, mt0 + mt, :]
                        )
                    return
                if no_transpose:
                    # sim-only: skip PE transposes, model DMA traffic directly
                    for j in range(NS // P):
                        for mt in range(nmt):
                            nc.sync.dma_start(
                                xown_dram[j * P:(j + 1) * P, mt * P:(mt + 1) * P],
                                xT[:, mt0 + mt, j * P:(j + 1) * P],
                            )
                    return
                for j in range(NS // P):
                    xo = xo_p.tile([P, nmt, P], f16, tag="xo")
                    for mt in range(nmt):
                        pt = ps_p.tile([P, P], f16, tag="mm")
                        nc.tensor.transpose(pt[:], xT[:, mt0 + mt, j * P:(j + 1) * P], ident[:])
                        if drain_split and mt % 2 == 1:
                            nc.scalar.copy(xo[:, mt, :], pt[:])
                        else:
                            nc.vector.tensor_copy(xo[:, mt, :], pt[:])
                    nc.sync.dma_start(xown_dram[j * P:(j + 1) * P, :], xo[:])

            uT0 = big_p.tile([P, T // P, NS], f16, tag="uT")
            hT = {}
            xT = {}

            # ---- layer 0 ----
            def x0_load(xs, k, c0, w):
                nc.sync.dma_start(xs[:], x0_d[k, :, c0:c0 + w])

            hT[0] = big_p.tile([P, H // P, NS], f16, tag="hT", name="hT0")
            xT[0] = big_p.tile([P, H // P, NS], f16, tag="xT", name="xT0")
            half0 = range(0, H // P // 2)
            half1 = range(H // P // 2, H // P)
            if not l0_split:
                with nc.named_scope("l0_agg"):
                    agg(T, x0_load, uT0)
                with nc.named_scope("l0_lin1"):
                    linear(w_d["w1_0"], T // P, H // P, uT0, hT[0], bias_sb["b1_0"], relu=True)
                for h, mts in ((0, half0), (1, half1)):
                    with nc.named_scope(f"l0_lin2_{h}"):
                        linear(w_d["w2_0"], H // P, H // P, hT[0], xT[0], bias_sb["b2_0"],
                               relu=True, mts=mts)
                    with nc.named_scope(f"l0_tp_{h}"):
                        transpose_store(xT[0], xown[1, h], h)
                    with nc.named_scope(f"ag1_{h}"):
                        all_gather(1, h)
            if l0_split:
              # layer 0's agg is AT-stream-bound (55us of MMs vs 90us of DMA), so
              # interleave its n-halves with MLP compute to cover the streaming
              with nc.named_scope("l0_agg0"):
                agg(T, x0_load, uT0, ns=[0])
              with nc.named_scope("l0_lin1_0"):
                  linear(w_d["w1_0"], T // P, H // P, uT0, hT[0], bias_sb["b1_0"],
                         relu=True, ns=[0])
              with nc.named_scope("l0_lin2_h0n0"):
                  linear(w_d["w2_0"], H // P, H // P, hT[0], xT[0], bias_sb["b2_0"],
                         relu=True, mts=half0, ns=[0])
              with nc.named_scope("l0_agg1"):
                  agg(T, x0_load, uT0, ns=[1])
              with nc.named_scope("l0_lin1_1"):
                  linear(w_d["w1_0"], T // P, H // P, uT0, hT[0], bias_sb["b1_0"],
                         relu=True, ns=[1])
              with nc.named_scope("l0_lin2_h0n1"):
                  linear(w_d["w2_0"], H // P, H // P, hT[0], xT[0], bias_sb["b2_0"],
                         relu=True, mts=half0, ns=[1])
              with nc.named_scope("l0_tp_0"):
                  transpose_store(xT[0], xown[1, 0], 0)
              with nc.named_scope("ag1_0"):
                  all_gather(1, 0)
              with nc.named_scope("l0_lin2_h1"):
                  linear(w_d["w2_0"], H // P, H // P, hT[0], xT[0], bias_sb["b2_0"],
                         relu=True, mts=half1)
              with nc.named_scope("l0_tp_1"):
                  transpose_store(xT[0], xown[1, 1], 1)
              with nc.named_scope("ag1_1"):
                  all_gather(1, 1)

            # ---- layers 1..3 ----
            for l in (1, 2, 3):
                uT = big_p.tile([P, H // P, NS], f16, tag="uT", name=f"uT{l}")
                with nc.named_scope(f"l{l}_agg"):
                    g0h, g1h = xg[l, 0], xg[l, 1]

                    def x_load(xs, k, c0, w, g0h=g0h, g1h=g1h):
                        gh = g0h if c0 < HH else g1h
                        c = c0 % HH
                        assert c + w <= HH
                        if dma_tp:
                            # xs[node, feat] <- XBAR-transposed [feat, node] block
                            r, j = k // (NS // P), k % (NS // P)
                            nc.sync.dma_start_transpose(
                                xs[:], gh[r * HH + c:r * HH + c + w, j * P:(j + 1) * P]
                            )
                        else:
                            nc.sync.dma_start(xs[:], gh[k * P:(k + 1) * P, c:c + w])

                    agg(H, x_load, uT)
                mt_out = (O if l == 3 else H) // P
                hT[l] = big_p.tile([P, mt_out, NS], f16, tag="hT", name=f"hTl{l}")
                with nc.named_scope(f"l{l}_lin1"):
                    linear(w_d[f"w1_{l}"], H // P, mt_out, uT, hT[l], bias_sb[f"b1_{l}"], relu=True)
                if l < 3:
                    xT[l] = big_p.tile([P, H // P, NS], f16, tag="xT", name=f"xTl{l}")
                    for h in (0, 1):
                        mts = range(h * (H // P // 2), (h + 1) * (H // P // 2))
                        with nc.named_scope(f"l{l}_lin2_{h}"):
                            linear(w_d[f"w2_{l}"], H // P, H // P, hT[l], xT[l],
                                   bias_sb[f"b2_{l}"], relu=True, mts=mts)
                        with nc.named_scope(f"l{l}_tp_{h}"):
                            transpose_store(xT[l], xown[l + 1, h], h)
                        with nc.named_scope(f"ag{l + 1}_{h}"):
                            all_gather(l + 1, h)

            # ---- fused heads ----
            mean_sb = const_p.tile([P, NS], f32, tag="mean_sb")
            var_sb = const_p.tile([P, NS], f32, tag="var_sb")
            z_sb = const_p.tile([P, NS], f32, tag="z_sb")
            with nc.named_scope("heads"):
                for W_sb, b_sb, o_sb in ((whm_sb, bhm_sb, mean_sb), (whv_sb, bhv_sb, var_sb)):
                    for n in range(ND):
                        p = ps_p.tile([P, 512], f32, tag="mm")
                        for k in range(O // P):
                            nc.tensor.matmul(
                                p[:],
                                lhsT=W_sb[:, k, :],
                                rhs=hT[3][:, k, n * 512:(n + 1) * 512],
                                start=(k == 0),
                                stop=(k == O // P - 1),
                            )
                        nc.scalar.activation(
                            o_sb[:, n * 512:(n + 1) * 512], p[:], AF.Identity,
                            bias=b_sb[:, 0:1],
                        )
                nc.vector.tensor_tensor(z_sb[:], var_sb[:], eps_sb[:], mybir.AluOpType.mult)
                nc.vector.tensor_tensor(z_sb[:], z_sb[:], mean_sb[:], mybir.AluOpType.add)
                nc.sync.dma_start(mean_d[:], mean_sb[:])
                nc.sync.dma_start(var_d[:], var_sb[:])
                nc.sync.dma_start(z_d[:], z_sb[:])

    nc.compile()
    return nc


def _tile_lhsT(w):
    """[K, M] fp16 -> [Mt, 128, Kt, 128]; slab [mt] is SBUF-ready [128p, Kt, 128m]."""
    K, M = w.shape
    Kt, Mt = K // P, M // P
    return np.ascontiguousarray(w.reshape(Kt, P, Mt, P).transpose(2, 1, 0, 3))


def _bias_t(b):
    """[M] fp32 -> [128, Mt] (partition = feature within tile)."""
    return np.ascontiguousarray(b.reshape(-1, P).T).astype(np.float32)


def prepare_inputs(inputs):
    """Host-side preprocessing: adjacency build + layout tiling. Returns in_maps."""
    f16 = np.float16
    eeg_nodes = np.asarray(inputs["eeg_nodes"], np.float32)
    eeg_idx = np.asarray(inputs["eeg_idx"])
    src = eeg_idx[0].astype(np.int64)
    dst = eeg_idx[1].astype(np.int64)

    counts = np.bincount(src * N + dst, minlength=N * N).reshape(N, N)
    AT = counts.astype(np.float32)
    AT[np.arange(N), np.arange(N)] += 1.0  # fold GIN's (1+eps)*x self-term, eps=0
    AT16 = AT.astype(f16)
    del AT, counts

    # Activations explode to ~1.3e5 by layer 3 (> fp16 max).  Since relu is
    # positively homogeneous, scale each of layers 0-2's output by S=1/16
    # (exact power of 2), folded into w2/b2; heads unscale via x S^-3.
    S = np.float32(1.0 / 16.0)
    c = [np.float32(1.0), S, S * S, S * S * S]  # cumulative scale of x_l input

    common = {}
    common["x0"] = np.ascontiguousarray(eeg_nodes.astype(f16).reshape(KT_NODES, P, T))
    for l in range(4):
        common[f"w1_{l}"] = _tile_lhsT(np.asarray(inputs[f"w1_{l}"], np.float32).astype(f16))
        common[f"b1_{l}"] = _bias_t(np.asarray(inputs[f"b1_{l}"], np.float32) * c[l])
    for l in range(3):
        common[f"w2_{l}"] = _tile_lhsT((np.asarray(inputs[f"w2_{l}"], np.float32) * S).astype(f16))
        common[f"b2_{l}"] = _bias_t(np.asarray(inputs[f"b2_{l}"], np.float32) * c[l + 1])

    # fused heads:  mean = h3 @ (w2_3 @ wm) + (b2_3 @ wm + bm); h3 arrives
    # scaled by c[3] so the fused weight is unscaled by 1/c[3].
    w2_3 = np.asarray(inputs["w2_3"], np.float32)
    b2_3 = np.asarray(inputs["b2_3"], np.float32)
    wm = np.asarray(inputs["wm"], np.float32)
    wv = np.asarray(inputs["wv"], np.float32)
    W2m = ((w2_3 @ wm) / c[3]).astype(f16)
    W2v = ((w2_3 @ wv) / c[3]).astype(f16)
    common["whm"] = _tile_lhsT(W2m)[0]
    common["whv"] = _tile_lhsT(W2v)[0]
    common["bhm"] = (b2_3 @ wm + np.asarray(inputs["bm"], np.float32)).reshape(P, 1).astype(np.float32)
    common["bhv"] = (b2_3 @ wv + np.asarray(inputs["bv"], np.float32)).reshape(P, 1).astype(np.float32)

    eps = np.asarray(inputs["eps"], np.float32)
    in_maps = []
    for c in range(NC):
        m = dict(common)
        blk = AT16[:, c * NS:(c + 1) * NS]
        m["at_t"] = np.ascontiguousarray(
            blk.reshape(KT_NODES, P, ND, 512).transpose(0, 2, 1, 3)
        )
        m["epst"] = np.ascontiguousarray(eps[c * NS:(c + 1) * NS, :].T)
        in_maps.append(m)
    return in_maps


def get_program():
    if "nc" not in _PROGRAM_CACHE:
        _PROGRAM_CACHE["nc"] = _build_program()
    return _PROGRAM_CACHE["nc"]


def assemble_outputs(results):
    z = np.empty((N, L), np.float32)
    mean = np.empty((N, L), np.float32)
    var = np.empty((N, L), np.float32)
    for c in range(NC):
        z[c * NS:(c + 1) * NS] = results[c]["zt"].T
        mean[c * NS:(c + 1) * NS] = results[c]["meant"].T
        var[c * NS:(c + 1) * NS] = results[c]["vart"].T
    return z, mean, var


def kernel(**inputs):
    from concourse.bass_utils import run_bass_kernel_spmd

    nc = get_program()
    in_maps = prepare_inputs(inputs)
    res = run_bass_kernel_spmd(nc, in_maps, core_ids=list(range(NC)))
    return assemble_outputs(res.results)



# revision 24
# speedup vs baseline: 1.7192x; 1.7192x over previous
"""Trainium2 Bass kernel for nn_DemLocGraphEncoder (4-layer GIN + variational heads).

Strategy (v2: fp8 DoubleRow aggregation + chunk-pipelined AllGathers)
--------------------------------------------------------------------
The GIN segment-sum is a dense matmul against the host-built (I+A)^T
multiplicity matrix.  AT entries are small integers (<=3) => exact in fp8
e4m3; the aggregated activations are shipped and multiplied in fp8 e4m3
with per-layer power-of-2 scaling (empirical end-to-end rel err ~6e-3 vs
the 2e-2 budget).  This enables:
  * the AT shard resident in SBUF as fp8 (8.4 MB, loaded once — no HBM
    re-streaming across the 4 layers),
  * DoubleRow fp8 matmuls for the aggregation (2 k-tiles per instruction),
  * AllGather payloads in fp8 (half the wire bytes of fp16).
Layer 3 pulls W1_3 in front of the gather (agg(x)@W1 == agg(x@W1)): the
last AllGather carries 1024-wide y3 instead of 2048-wide x3 and layer 3's
aggregation drains straight through relu+bias into h3; w2_3@{wm,wv} fold
into two fused [1024,128] heads.

Pipelining: nodes are row-sharded 1024/core; x_{l+1} is produced in NCH
node-chunks (lin1/lin2 of a chunk emitted as soon as the aggregation
columns it needs are drained), each chunk AllGathered separately so wire
time hides under remaining layer compute.  The next layer's aggregation
consumes chunks in k-passes (one per source chunk), accumulating partials
in SBUF (pass-0 copy, later passes DVE-add from PSUM).  Source rows of AT
(and x0) are host-permuted into gathered order (chunk-major, then rank).

The MLP stays fp16 (fp8 weights fail the accuracy budget).  Per-layer
scales fold into W1/W2/b1/b2 and the heads host-side; activations quantize
to fp8 for free in the lin2 PSUM->SBUF activation drain.
"""

import sys

if "/opt/trn_rl_repo" not in sys.path:
    sys.path.insert(0, "/opt/trn_rl_repo")

import numpy as np

N, E, T, H, O, L = 8192, 262144, 256, 2048, 1024, 128
NC = 8
NS = N // NC          # 1024 nodes per core
P = 128
KK = N // 256         # 32 DoubleRow k-pairs over source nodes

DEFAULT_NCH = 2

# per-layer power-of-2 scales for fp8 payloads (max|x_l| = 5.1, 26.8, 266,
# 4011; y3 max 3272 -> scaled maxima ~100, safely under TRN e4m3's 240)
S0, S1, S2, S3, SY = 16.0, 4.0, 0.25, 1.0 / 64.0, 1.0 / 32.0

_PROGRAM_CACHE = {}


def _build_program(collectives=True, opts=None):
    opts = dict(opts or {})
    NCH = opts.get("nch", DEFAULT_NCH)
    w_bufs = opts.get("w_bufs", 3)
    xs_bufs = opts.get("xs_bufs", 3)
    ps_bufs = opts.get("ps_bufs", 8)
    drain_split = opts.get("drain_split", True)
    use_dr = opts.get("dr", True)

    import concourse.bass as bass  # noqa: F401
    import concourse.mybir as mybir
    import concourse.tile as tile
    from concourse import bacc
    from concourse.masks import make_identity

    f8 = mybir.dt.float8e4
    f16 = mybir.dt.float16
    f32 = mybir.dt.float32
    AF = mybir.ActivationFunctionType
    DR = mybir.MatmulPerfMode.DoubleRow

    CN = NS // NCH                    # nodes per chunk per core
    KKC = KK // NCH                   # k-pairs per source chunk
    CP = CN // 256                    # k-pairs per (chunk, rank)
    assert CN % 256 == 0, "chunk must hold whole DoubleRow pairs"

    nc = bacc.Bacc(
        "TRN2", target_bir_lowering=False, debug=False,
        num_devices=NC if collectives else 1,
    )

    # ---- I/O ----
    # partition-major so the single resident-load DMA is dim-matched with at_sb
    at_d = nc.dram_tensor("at8", [P, KK, 2, NS], f8, kind="ExternalInput")
    x0_d = nc.dram_tensor("x08", [KK, P, 2, T], f8, kind="ExternalInput")
    w_d = {}
    w_d["w1_0"] = nc.dram_tensor("w1_0", [H // P, P, T // P, P], f16, kind="ExternalInput")
    w_d["w2_0"] = nc.dram_tensor("w2_0", [H // P, P, H // P, P], f16, kind="ExternalInput")
    for l in (1, 2):
        w_d[f"w1_{l}"] = nc.dram_tensor(f"w1_{l}", [H // P, P, H // P, P], f16, kind="ExternalInput")
        w_d[f"w2_{l}"] = nc.dram_tensor(f"w2_{l}", [H // P, P, H // P, P], f16, kind="ExternalInput")
    w_d["w1_3"] = nc.dram_tensor("w1_3", [O // P, P, H // P, P], f16, kind="ExternalInput")
    whm_d = nc.dram_tensor("whm", [P, O // P, P], f16, kind="ExternalInput")
    whv_d = nc.dram_tensor("whv", [P, O // P, P], f16, kind="ExternalInput")
    b_d = {}
    for l in range(3):
        b_d[f"b1_{l}"] = nc.dram_tensor(f"b1_{l}", [P, H // P], f32, kind="ExternalInput")
        b_d[f"b2_{l}"] = nc.dram_tensor(f"b2_{l}", [P, H // P], f32, kind="ExternalInput")
    b_d["b1_3"] = nc.dram_tensor("b1_3", [P, O // P], f32, kind="ExternalInput")
    bhm_d = nc.dram_tensor("bhm", [P, 1], f32, kind="ExternalInput")
    bhv_d = nc.dram_tensor("bhv", [P, 1], f32, kind="ExternalInput")
    eps_d = nc.dram_tensor("epst", [P, NS], f32, kind="ExternalInput")

    z_d = nc.dram_tensor("zt", [P, NS], f32, kind="ExternalOutput")
    mean_d = nc.dram_tensor("meant", [P, NS], f32, kind="ExternalOutput")
    var_d = nc.dram_tensor("vart", [P, NS], f32, kind="ExternalOutput")
    debug = opts.get("debug", False)
    if debug:
        dbg = {
            "d_u0": nc.dram_tensor("d_u0", [P, 2, 512], f16, kind="ExternalOutput"),
            "d_xg": nc.dram_tensor("d_xg", [P, 2, 512], f16, kind="ExternalOutput"),
            "d_u1": nc.dram_tensor("d_u1", [P, H // P, 256], f16, kind="ExternalOutput"),
            "d_h3": nc.dram_tensor("d_h3", [P, O // P, 256], f16, kind="ExternalOutput"),
        }

    # per-boundary staging + gathered buffers, DoubleRow pair layout:
    # own [CP, P, 2, width]; gathered concatenates rank chunks on dim0.
    own = {}
    gath = {}
    for l, width in ((1, H), (2, H), (3, O)):
        for c in range(NCH):
            own[l, c] = nc.dram_tensor(f"own{l}_{c}", [CP, P, 2, width], f8)
            gath[l, c] = nc.dram_tensor(f"g{l}_{c}", [NC * CP, P, 2, width], f8,
                                        addr_space="Shared")

    rg = [list(range(NC))]

    with tile.TileContext(nc) as tc:
        with (
            tc.tile_pool(name="const", bufs=1) as const_p,
            tc.tile_pool(name="big", bufs=1) as big_p,
            tc.tile_pool(name="xs", bufs=xs_bufs) as xs_p,
            tc.tile_pool(name="w", bufs=w_bufs) as w_p,
            tc.tile_pool(name="xo", bufs=2) as xo_p,
            tc.tile_pool(name="stg", bufs=opts.get("stg_bufs", 2)) as stg_p,
            tc.tile_pool(name="stg3", bufs=1) as stg3_p,
            tc.tile_pool(name="ps", bufs=ps_bufs, space="PSUM") as ps_p,
        ):
            ident8 = const_p.tile([P, P], f8, tag="ident8")
            make_identity(nc, ident8)

            bias_sb = {}
            for name, d in b_d.items():
                bias_sb[name] = const_p.tile(list(d.shape), f32, tag=f"b_{name}", name=f"b_{name}")
                nc.sync.dma_start(bias_sb[name][:], d[:])
            bhm_sb = const_p.tile([P, 1], f32, tag="bhm")
            nc.sync.dma_start(bhm_sb[:], bhm_d[:])
            bhv_sb = const_p.tile([P, 1], f32, tag="bhv")
            nc.sync.dma_start(bhv_sb[:], bhv_d[:])
            eps_sb = const_p.tile([P, NS], f32, tag="eps")
            nc.sync.dma_start(eps_sb[:], eps_d[:])
            whm_sb = const_p.tile([P, O // P, P], f16, tag="whm")
            nc.sync.dma_start(whm_sb[:], whm_d[:])
            whv_sb = const_p.tile([P, O // P, P], f16, tag="whv")
            nc.sync.dma_start(whv_sb[:], whv_d[:])

            at_sb = const_p.tile([P, KK, 2, NS], f8, tag="at8")
            nc.sync.dma_start(at_sb[:], at_d[:])

            def at_rhs(kk, ng):
                return at_sb[:, kk, :, ng * 512:(ng + 1) * 512]

            def all_gather(l, c):
                if collectives:
                    nc.gpsimd.collective_compute(
                        "AllGather", mybir.AluOpType.bypass, replica_groups=rg,
                        ins=[own[l, c][:].opt()], outs=[gath[l, c][:].opt()],
                    )
                else:
                    for r in range(NC):
                        nc.sync.dma_start(gath[l, c][r * CP:(r + 1) * CP], own[l, c][:])

            def x0_load(kk, g):
                xs = xs_p.tile([P, 2, T], f8, tag="xs", name="xs0")
                nc.sync.dma_start(xs[:], x0_d[kk])
                return xs

            def mk_gath_load(l, width):
                gw = min(width, 1024)

                def load(kk, g):
                    c_src, gpair = divmod(kk, KKC)
                    xs = xs_p.tile([P, 2, gw], f8, tag="xs", name=f"xs{l}")
                    nc.sync.dma_start(
                        xs[:], gath[l, c_src][gpair, :, :, g * 1024:g * 1024 + gw]
                    )
                    return xs
                return load

            # ---------------- aggregation ----------------
            def agg_pass(uT, kks, x_load_fn, Mt, first):
                """One k-pass of the aggregation over source pairs `kks`.

                first: copy psums into uT; else DVE-add into uT.
                """
                ngroups = (Mt + 7) // 8
                for ng in range(2):
                    for g in range(ngroups):
                        mis = list(range(g * 8, min(Mt, (g + 1) * 8)))
                        psums = [ps_p.tile([P, 512], f32, tag="mm", name=f"ps{i}")
                                 for i in range(len(mis))]
                        for ki, kk in enumerate(kks):
                            xs = x_load_fn(kk, g)
                            for i, mi in enumerate(mis):
                                ms = slice((mi - g * 8) * P, (mi - g * 8 + 1) * P)
                                if use_dr:
                                    nc.tensor.matmul(
                                        psums[i][:],
                                        lhsT=xs[:, :, ms], rhs=at_rhs(kk, ng),
                                        start=(ki == 0), stop=(ki == len(kks) - 1),
                                        perf_mode=DR,
                                    )
                                else:
                                    for pi in range(2):
                                        nc.tensor.matmul(
                                            psums[i][:],
                                            lhsT=xs[:, pi, ms],
                                            rhs=at_sb[:, kk, pi, ng * 512:(ng + 1) * 512],
                                            start=(ki == 0 and pi == 0),
                                            stop=(ki == len(kks) - 1 and pi == 1),
                                        )
                        for i, mi in enumerate(mis):
                            dst = uT[:, mi, ng * 512:(ng + 1) * 512]
                            if first:
                                if drain_split and i % 2 == 1:
                                    nc.scalar.copy(dst, psums[i][:])
                                else:
                                    nc.vector.tensor_copy(dst, psums[i][:])
                            else:
                                nc.vector.tensor_tensor(dst, dst, psums[i][:],
                                                        mybir.AluOpType.add)

            # ---------------- fp16 linear ----------------
            def linear(w_dram, Kt, Mt, rhsT, outT, bias, relu, c0, cw, staged_out=False):
                """outT[:, mt, cols] = act(sum_k w[mt,k]^T @ rhsT[:, k, c0:c0+cw])."""
                for mt in range(Mt):
                    ws = w_p.tile([P, Kt, P], f16, tag="w")
                    nc.sync.dma_start(ws[:], w_dram[mt])
                    p = ps_p.tile([P, cw], f32, tag="mm")
                    for k in range(Kt):
                        nc.tensor.matmul(
                            p[:], lhsT=ws[:, k, :], rhs=rhsT[:, k, c0:c0 + cw],
                            start=(k == 0), stop=(k == Kt - 1),
                        )
                    dst = outT[:, mt, :] if staged_out else outT[:, mt, c0:c0 + cw]
                    nc.scalar.activation(
                        dst, p[:], AF.Relu if relu else AF.Identity,
                        bias=bias[:, mt:mt + 1] if bias is not None else 0.0,
                    )

            def transpose_store(srcT, Mt, own_dram):
                """srcT [P, Mt, CN] fp8 chunk -> own_dram [CP, P, 2, Mt*P]."""
                for j in range(CN // P):
                    xo = xo_p.tile([P, Mt, P], f8, tag="xo")
                    for mt in range(Mt):
                        # fp8 transpose requires output element step 2 in PSUM
                        pt = ps_p.tile([P, P, 2], f8, tag="mm")
                        nc.tensor.transpose(pt[:, :, 0], srcT[:, mt, j * P:(j + 1) * P], ident8[:])
                        if drain_split and mt % 2 == 1:
                            nc.scalar.copy(xo[:, mt, :], pt[:, :, 0])
                        else:
                            nc.vector.tensor_copy(xo[:, mt, :], pt[:, :, 0])
                    nc.sync.dma_start(own_dram[j // 2, :, j % 2, :], xo[:])

            # fresh pool tiles per layer (same tag, bufs=1): pool rotation
            # inserts the write-after-read edges when a layer's aggregation
            # drains replace the previous layer's uT/hT
            uT = big_p.tile([P, H // P, NS], f16, tag="uT", name="uT0")
            hT = big_p.tile([P, H // P, NS], f16, tag="hT", name="hT0")

            # ================ layer 0 ================
            with nc.named_scope("l0_agg"):
                agg_pass(uT, list(range(KK)), x0_load, T // P, first=True)
            if debug:
                cp_u0 = const_p.tile([P, 2, 512], f16, tag="cp_u0")
                nc.vector.tensor_copy(cp_u0[:], uT[:, 0:2, 0:512])
            for c in range(NCH):
                with nc.named_scope(f"l0_lin1_c{c}"):
                    linear(w_d["w1_0"], T // P, H // P, uT, hT, bias_sb["b1_0"],
                           relu=True, c0=c * CN, cw=CN)
                xstg = stg_p.tile([P, H // P, CN], f8, tag="xstg")
                with nc.named_scope(f"l0_lin2_c{c}"):
                    linear(w_d["w2_0"], H // P, H // P, hT, xstg, bias_sb["b2_0"],
                           relu=True, c0=c * CN, cw=CN, staged_out=True)
                with nc.named_scope(f"l0_tp_c{c}"):
                    transpose_store(xstg, H // P, own[1, c])
                with nc.named_scope(f"ag1_{c}"):
                    all_gather(1, c)
            if debug:
                # tile load of gathered payload (RAW-ordered vs the collective),
                # upcast to f16 via engine copy into a never-rewritten tile
                xg_t = xs_p.tile([P, 2, 512], f8, tag="xs", name="xg_dbg")
                nc.sync.dma_start(xg_t[:], gath[1, 0][0, :, :, 0:512])
                cp_xg = const_p.tile([P, 2, 512], f16, tag="cp_xg")
                nc.vector.tensor_copy(cp_xg[:], xg_t[:])

            # ================ layers 1, 2 ================
            fuse_q = opts.get("fuse_q", False)
            for l in (1, 2):
                uT = big_p.tile([P, H // P, NS], f16, tag="uT", name=f"uT{l}")
                hT = big_p.tile([P, H // P, NS], f16, tag="hT", name=f"hT{l}")
                xg_load = mk_gath_load(l, H)
                if fuse_q:
                    with nc.named_scope(f"l{l}_agg"):
                        agg_pass(uT, list(range(KK)), xg_load, H // P, first=True)
                else:
                    for q in range(NCH):
                        with nc.named_scope(f"l{l}_agg_q{q}"):
                            agg_pass(uT, list(range(q * KKC, (q + 1) * KKC)), xg_load,
                                     H // P, first=(q == 0))
                if debug and l == 1:
                    cp_u1 = const_p.tile([P, H // P, 256], f16, tag="cp_u1")
                    nc.vector.tensor_copy(cp_u1[:], uT[:, :, 0:256])
                for c in range(NCH):
                    with nc.named_scope(f"l{l}_lin1_c{c}"):
                        linear(w_d[f"w1_{l}"], H // P, H // P, uT, hT, bias_sb[f"b1_{l}"],
                               relu=True, c0=c * CN, cw=CN)
                    if l == 1:
                        xstg = stg_p.tile([P, H // P, CN], f8, tag="xstg")
                        with nc.named_scope(f"l1_lin2_c{c}"):
                            linear(w_d["w2_1"], H // P, H // P, hT, xstg, bias_sb["b2_1"],
                                   relu=True, c0=c * CN, cw=CN, staged_out=True)
                        with nc.named_scope(f"l1_tp_c{c}"):
                            transpose_store(xstg, H // P, own[2, c])
                        with nc.named_scope(f"ag2_{c}"):
                            all_gather(2, c)
                    else:
                        x3stg = stg3_p.tile([P, H // P, CN], f16, tag="x3stg")
                        with nc.named_scope(f"l2_lin2_c{c}"):
                            linear(w_d["w2_2"], H // P, H // P, hT, x3stg, bias_sb["b2_2"],
                                   relu=True, c0=c * CN, cw=CN, staged_out=True)
                        ystg = stg_p.tile([P, O // P, CN], f8, tag="ystg")
                        with nc.named_scope(f"y3_c{c}"):
                            linear(w_d["w1_3"], H // P, O // P, x3stg, ystg, None,
                                   relu=False, c0=0, cw=CN, staged_out=True)
                        with nc.named_scope(f"l2_tp_c{c}"):
                            transpose_store(ystg, O // P, own[3, c])
                        with nc.named_scope(f"ag3_{c}"):
                            all_gather(3, c)

            # ================ layer 3 ================
            yg_load = mk_gath_load(3, O)
            h3T = big_p.tile([P, O // P, NS], f16, tag="hT", name="h3T")
            with nc.named_scope("l3_agg"):
                for ng in range(2):
                    psums = [ps_p.tile([P, 512], f32, tag="mm", name=f"ps{i}")
                             for i in range(8)]
                    for kk in range(KK):
                        xs = yg_load(kk, 0)
                        for mi in range(8):
                            if use_dr:
                                nc.tensor.matmul(
                                    psums[mi][:],
                                    lhsT=xs[:, :, mi * P:(mi + 1) * P],
                                    rhs=at_rhs(kk, ng),
                                    start=(kk == 0), stop=(kk == KK - 1),
                                    perf_mode=DR,
                                )
                            else:
                                for pi in range(2):
                                    nc.tensor.matmul(
                                        psums[mi][:],
                                        lhsT=xs[:, pi, mi * P:(mi + 1) * P],
                                        rhs=at_sb[:, kk, pi, ng * 512:(ng + 1) * 512],
                                        start=(kk == 0 and pi == 0),
                                        stop=(kk == KK - 1 and pi == 1),
                                    )
                    for mi in range(8):
                        nc.scalar.activation(
                            h3T[:, mi, ng * 512:(ng + 1) * 512], psums[mi][:],
                            AF.Relu, bias=bias_sb["b1_3"][:, mi:mi + 1],
                        )

            # ---- fused heads ----
            mean_sb = const_p.tile([P, NS], f32, tag="mean_sb")
            var_sb = const_p.tile([P, NS], f32, tag="var_sb")
            z_sb = eps_sb  # eps is dead after z = mean + var*eps folds it in
            with nc.named_scope("heads"):
                for W_sb, b_sb, o_sb in ((whm_sb, bhm_sb, mean_sb), (whv_sb, bhv_sb, var_sb)):
                    for n in range(2):
                        p = ps_p.tile([P, 512], f32, tag="mm")
                        for k in range(O // P):
                            nc.tensor.matmul(
                                p[:], lhsT=W_sb[:, k, :],
                                rhs=h3T[:, k, n * 512:(n + 1) * 512],
                                start=(k == 0), stop=(k == O // P - 1),
                            )
                        nc.scalar.activation(
                            o_sb[:, n * 512:(n + 1) * 512], p[:], AF.Identity,
                            bias=b_sb[:, 0:1],
                        )
                nc.vector.tensor_tensor(z_sb[:], var_sb[:], eps_sb[:], mybir.AluOpType.mult)
                nc.vector.tensor_tensor(z_sb[:], z_sb[:], mean_sb[:], mybir.AluOpType.add)
                nc.sync.dma_start(mean_d[:], mean_sb[:])
                nc.sync.dma_start(var_d[:], var_sb[:])
                nc.sync.dma_start(z_d[:], z_sb[:])
            if debug:
                nc.sync.dma_start(dbg["d_u0"][:], cp_u0[:])
                nc.sync.dma_start(dbg["d_xg"][:], cp_xg[:])
                nc.sync.dma_start(dbg["d_u1"][:], cp_u1[:])
                cp_h3 = const_p.tile([P, O // P, 256], f16, tag="cp_h3")
                nc.vector.tensor_copy(cp_h3[:], h3T[:, :, 0:256])
                nc.sync.dma_start(dbg["d_h3"][:], cp_h3[:])

    nc.compile()
    return nc


def _tile_lhsT(w):
    """[K, M] fp16 -> [Mt, 128, Kt, 128]; slab [mt] is SBUF-ready [128p, Kt, 128m]."""
    K, M = w.shape
    Kt, Mt = K // P, M // P
    return np.ascontiguousarray(w.reshape(Kt, P, Mt, P).transpose(2, 1, 0, 3))


def _bias_t(b):
    """[M] fp32 -> [128, Mt] (partition = feature within tile)."""
    return np.ascontiguousarray(b.reshape(-1, P).T).astype(np.float32)


def _to_f8(x):
    import ml_dtypes
    return np.clip(x, -240.0, 240.0).astype(ml_dtypes.float8_e4m3fn)


def _dr_tiles(x):
    """[n_rows, W] (rows already in gathered order) -> [n_rows//256, 128, 2, W]."""
    n, w = x.shape
    return np.ascontiguousarray(x.reshape(n // 256, 2, P, w).transpose(0, 2, 1, 3))


def _src_perm(nch):
    """Gathered source-row order: chunk-major, then rank, then node."""
    cn = NS // nch
    return np.concatenate([
        np.arange(cn) + r * NS + c * cn
        for c in range(nch) for r in range(NC)
    ])


def prepare_inputs(inputs, nch=DEFAULT_NCH):
    """Host-side preprocessing: adjacency build + layout tiling + scale folding."""
    f16 = np.float16
    eeg_nodes = np.asarray(inputs["eeg_nodes"], np.float32)
    eeg_idx = np.asarray(inputs["eeg_idx"])
    src = eeg_idx[0].astype(np.int64)
    dst = eeg_idx[1].astype(np.int64)

    counts = np.bincount(src * N + dst, minlength=N * N).reshape(N, N)
    AT = counts.astype(np.float32)
    AT[np.arange(N), np.arange(N)] += 1.0  # fold GIN's (1+eps)*x self-term
    perm = _src_perm(nch)
    AT = AT[perm]          # source rows into gathered order
    AT8 = _to_f8(AT)
    del AT, counts

    s = [np.float32(v) for v in (S0, S1, S2, S3)]
    sy = np.float32(SY)

    common = {}
    common["x08"] = _dr_tiles(_to_f8(eeg_nodes[perm] * s[0]))
    for l in range(3):
        w1 = np.asarray(inputs[f"w1_{l}"], np.float32)
        common[f"w1_{l}"] = _tile_lhsT((w1 / s[l]).astype(f16))
        common[f"b1_{l}"] = _bias_t(np.asarray(inputs[f"b1_{l}"], np.float32))
        w2 = np.asarray(inputs[f"w2_{l}"], np.float32)
        common[f"w2_{l}"] = _tile_lhsT((w2 * s[l + 1]).astype(f16))
        common[f"b2_{l}"] = _bias_t(np.asarray(inputs[f"b2_{l}"], np.float32) * s[l + 1])
    # y3 = x3_scaled @ (W1_3 * sy/s3); b1_3 applied post-agg on sy scale
    common["w1_3"] = _tile_lhsT((np.asarray(inputs["w1_3"], np.float32) * (sy / s[3])).astype(f16))
    common["b1_3"] = _bias_t(np.asarray(inputs["b1_3"], np.float32) * sy)

    # fused heads: h3 arrives scaled by sy -> unscale inside the fused weight
    w2_3 = np.asarray(inputs["w2_3"], np.float32)
    b2_3 = np.asarray(inputs["b2_3"], np.float32)
    wm = np.asarray(inputs["wm"], np.float32)
    wv = np.asarray(inputs["wv"], np.float32)
    common["whm"] = _tile_lhsT(((w2_3 @ wm) / sy).astype(f16))[0]
    common["whv"] = _tile_lhsT(((w2_3 @ wv) / sy).astype(f16))[0]
    common["bhm"] = (b2_3 @ wm + np.asarray(inputs["bm"], np.float32)).reshape(P, 1).astype(np.float32)
    common["bhv"] = (b2_3 @ wv + np.asarray(inputs["bv"], np.float32)).reshape(P, 1).astype(np.float32)

    eps = np.asarray(inputs["eps"], np.float32)
    in_maps = []
    for c in range(NC):
        m = dict(common)
        m["at8"] = np.ascontiguousarray(
            _dr_tiles(AT8[:, c * NS:(c + 1) * NS]).transpose(1, 0, 2, 3))
        m["epst"] = np.ascontiguousarray(eps[c * NS:(c + 1) * NS, :].T)
        in_maps.append(m)
    return in_maps


def get_program(opts=None):
    key = repr(opts)
    if key not in _PROGRAM_CACHE:
        _PROGRAM_CACHE[key] = _build_program(opts=opts)
    return _PROGRAM_CACHE[key]


def assemble_outputs(results):
    z = np.empty((N, L), np.float32)
    mean = np.empty((N, L), np.float32)
    var = np.empty((N, L), np.float32)
    for c in range(NC):
        z[c * NS:(c + 1) * NS] = results[c]["zt"].T
        mean[c * NS:(c + 1) * NS] = results[c]["meant"].T
        var[c * NS:(c + 1) * NS] = results[c]["vart"].T
    return z, mean, var


def kernel(**inputs):
    from concourse.bass_utils import run_bass_kernel_spmd

    nc = get_program()
    in_maps = prepare_inputs(inputs)
    res = run_bass_kernel_spmd(nc, in_maps, core_ids=list(range(NC)))
    return assemble_outputs(res.results)
